# revision 22
# baseline (speedup 1.0000x reference)
"""AAM attention block (B=4, C=256, H=W=64) on 8 TRN2 NeuronCores.

Sharding: data-parallel over batch (4) x sequence-parallel over query rows
(2) = 8 cores, zero collectives.  Each core's xn is host-permuted so ITS
query half occupies columns 0:2048 (softmax is key-order invariant).

v2 (fp8 PV): per-core program, fp16 energy / fp8 PV, fp32 PSUM accum:
  q = WqT.T @ xq + bq            [32x4, 2048]  (weights replicated on-chip)
  k = WkT.T @ xn + bk            [32x4, 4096]
  vT[n,c] = xn_sub.T @ WvT -> e4m3, packed [128, 32, 256] for DoubleRow
  per m-superblock of 512 query rows (pairs of 2x128 keys, one pair
  prefetched, cross-superblock too):
    eT = k_sub.T @ q_blk         fp16, quad-packed 32-row PE tiles
    exp8 = Exp(4*eT - 7)         ScalarE -> float8e5 (e5m2)
    out2[ch] += vt8.T @ exp8     fp8 DoubleRow (256-key contraction)
    s_ps += ones8.T @ exp8       fp8 DoubleRow, [1,512] PSUM = softmax denom
  tails (in next superblock's loop, interleaved across p positions):
    1/s = exp(-ln s); y = (Wo1T.T @ out2sb) * inv + (Wo2T.T @ xq + cvec)
    with cvec = bo + Wo1T.T @ bv; inv broadcast via K=1 ones matmul.
  Input DMAs issue in parallel from Sync/GpSimd/Scalar queues; final
  output DMAs issue from two queues.
"""

import json
import sys

import numpy as np

try:
    import concourse  # noqa: F401
except ImportError:  # pragma: no cover
    sys.path.insert(0, "/opt/trn_rl_repo")

C = 256
CQK = 32
N = 4096          # key/value positions per batch (64*64)
M = 2048          # query rows per core (N/2)
SB = 512          # m-superblock size
NSB = M // SB     # 4 superblocks
NSUB = N // 128   # 32 n-subtiles
NP = NSUB // 2    # 16 n-pairs per superblock
# exp(e + EXP_BIAS), cancels in softmax.  e5m2 range is wide (max 57344,
# min subnormal 2^-16); data has max logit 13.7 -> exp max e^6.7=810. -7
# keeps the fp32 PSUM s-sums comfortable and matches the fp16 baseline.
EXP_BIAS = -7.0

MAX_WAITS = 1     # this container's walrus accepts 1 sync wait per instruction


def _split_waits_json(bir_bytes):
    """Hoist excess per-instruction sync waits onto preceding same-engine NoOps."""
    j = json.loads(bir_bytes)
    uid = 0
    changed = False
    for fnx in j["functions"]:
        for b in fnx["blocks"]:
            newlist = []
            for ins in b["instructions"]:
                si = ins.get("sync_info") or {}
                ow = si.get("on_wait") or []
                if len(ow) > MAX_WAITS:
                    changed = True
                    extra, keep = ow[:-MAX_WAITS], ow[-MAX_WAITS:]
                    si["on_wait"] = keep
                    for i in range(0, len(extra), MAX_WAITS):
                        uid += 1
                        newlist.append({
                            "debug": ins.get("debug"),
                            "engine": ins["engine"],
                            "ins": [], "outs": [],
                            "name": f"WSPLIT-{uid}",
                            "opcode": "NoOp",
                            "sync_info": {"on_update": [],
                                          "on_wait": extra[i:i + MAX_WAITS]},
                        })
                newlist.append(ins)
            b["instructions"] = newlist
    return json.dumps(j).encode() if changed else bir_bytes


def _install_wait_split():
    import concourse.bass_utils as bu
    import concourse.bass2jax as b2j

    if getattr(bu, "_wait_split_installed", False):
        return
    orig = bu.compile_bir_kernel

    def patched(bir_json, tmpdir, neff_name="file.neff"):
        if isinstance(bir_json, str):
            bir_json = bir_json.encode()
        return orig(_split_waits_json(bir_json), tmpdir, neff_name=neff_name)

    bu.compile_bir_kernel = patched
    bu._wait_split_installed = True
    b2j.compile_bir_kernel = patched


def _build_nc():
    from contextlib import ExitStack

    import concourse.bass as bass
    import concourse.tile as tile
    from concourse import mybir

    f16 = mybir.dt.float16
    f32 = mybir.dt.float32
    f8e4 = mybir.dt.float8e4
    f8e5 = mybir.dt.float8e5
    DR = mybir.MatmulPerfMode.DoubleRow
    Exp = mybir.ActivationFunctionType.Exp
    Ln = mybir.ActivationFunctionType.Ln

    nc = bass.Bass()
    xn = nc.declare_dram_parameter("xn", [C, N], f16, isOutput=False)
    # wpA [128, 128] f16: wq0u|wq1u|wk0u|wk1u, each [128, 32] (unreplicated)
    wpA = nc.declare_dram_parameter("wpA", [128, 128], f16, isOutput=False)
    # wpB [128, 1536] f16: wv0|wv1 (256 each) | wo0..wo3 (256 each)
    wpB = nc.declare_dram_parameter("wpB", [128, 1536], f16, isOutput=False)
    # biases packed into one [128, 8] f32 DMA: bq|bk|bv0|bv1|bo0|bo1|pad
    bpack = nc.declare_dram_parameter("bpack", [128, 8], f32, isOutput=False)
    # cvec[cho] = bo + Wo1.T @ bv, folded into yx via a K=1 matmul
    cpack = nc.declare_dram_parameter("cpack", [1, 256], f16, isOutput=False)
    out = nc.declare_dram_parameter("out", [C, M], f16, isOutput=True)

    with tile.TileContext(nc) as tc, ExitStack() as ctx:
        consts = ctx.enter_context(tc.tile_pool(name="consts", bufs=1))
        big = ctx.enter_context(tc.tile_pool(name="big", bufs=1))
        expp = ctx.enter_context(tc.tile_pool(name="expp", bufs=8))
        scp = ctx.enter_context(tc.tile_pool(name="scp", bufs=4))
        yp = ctx.enter_context(tc.tile_pool(name="yp", bufs=3))
        # PSUM budget (8 banks): e pairs 2x[128,2,512]f32 = 4, out2 2x[128,512]
        # = 2, ps (s_ps [1,512] / inv_bc [128,512] alternating) = 1, pe misc = 1
        pe2 = ctx.enter_context(tc.tile_pool(name="pe2", bufs=2, space="PSUM"))
        pe_pool = ctx.enter_context(tc.tile_pool(name="pe", bufs=1, space="PSUM"))
        pacc = ctx.enter_context(tc.tile_pool(name="pacc", bufs=2, space="PSUM"))
        ps_pool = ctx.enter_context(tc.tile_pool(name="ps", bufs=1, space="PSUM"))

        # ---- constants (warm first: the PE warm-up gates on it) ----
        warm = consts.tile([128, 512], f16, name="warm")
        nc.vector.memset(warm, 0.0)
        ones_row = consts.tile([1, SB], f16, name="ones_row")
        nc.vector.memset(ones_row, 1.0)
        ones128f = consts.tile([128, 128], f16, name="ones128f")
        nc.vector.memset(ones128f, 1.0)
        # broadcast-ones stationary: s matmul writes s to ALL 128 partitions,
        # which doubles as the inv broadcast (no separate K=1 matmul needed)
        ones8 = consts.tile([128, 2, 128], f8e4, name="ones8")
        nc.vector.memset(ones8, 1.0)
        ebias = consts.tile([128, 1], f32, name="ebias")
        nc.vector.memset(ebias, EXP_BIAS)
        zbias = consts.tile([128, 1], f32, name="zbias")
        nc.vector.memset(zbias, 0.0)

        # ---- input DMAs, issued in parallel from three queues ----
        wpA_sb = consts.tile([128, 128], f16, name="wpA_sb")
        wqk = consts.tile([128, 512], f16, name="wqk")
        bp_sb = consts.tile([128, 8], f32, name="bp_sb")
        wpB_sb = consts.tile([128, 1536], f16, name="wpB_sb")
        cv_sb = consts.tile([1, 256], f16, name="cv_sb")
        xn0 = [[big.tile([128, 1024], f16, name=f"xn0_{i}_{c}")
                for c in range(2)] for i in range(2)]
        xn1 = [big.tile([128, 2048], f16, name=f"xn1_{i}") for i in range(2)]

        # critical-path inputs ride the Sync queue (hardware DGE); the
        # gpsimd/scalar queues are software-DGE with ~3us extra latency, so
        # they only carry late-need transfers (xn1 keys, wv/wo, cvec)
        nc.sync.dma_start(out=wpA_sb, in_=wpA[:, :])
        nc.sync.dma_start(out=xn0[0][0], in_=xn[0:128, 0:1024])
        nc.sync.dma_start(out=xn0[1][0], in_=xn[128:256, 0:1024])
        nc.gpsimd.dma_start(out=bp_sb, in_=bpack[:, :])
        nc.scalar.dma_start(out=wpB_sb, in_=wpB[:, :])
        nc.sync.dma_start(out=xn0[0][1], in_=xn[0:128, 1024:2048])
        nc.sync.dma_start(out=xn0[1][1], in_=xn[128:256, 1024:2048])
        nc.scalar.dma_start(out=cv_sb, in_=cpack[:, :])
        nc.gpsimd.dma_start(out=xn1[0], in_=xn[0:128, 2048:4096])
        nc.gpsimd.dma_start(out=xn1[1], in_=xn[128:256, 2048:4096])

        # on-chip 4x replication of wq/wk along the free dim:
        # wqk layout [wq0|wq1|wk0|wk1], 128 cols each = 4 x 32-col replicas
        for i in range(4):
            nc.vector.tensor_copy(wqk[:, 128 * i:128 * i + 32],
                                  wpA_sb[:, 32 * i:32 * i + 32])
        for i in range(4):
            nc.vector.tensor_copy(wqk[:, 128 * i + 32:128 * i + 64],
                                  wqk[:, 128 * i:128 * i + 32])
        for i in range(4):
            nc.vector.tensor_copy(wqk[:, 128 * i + 64:128 * i + 128],
                                  wqk[:, 128 * i:128 * i + 64])

        wq_sb = [wqk[:, 0:128], wqk[:, 128:256]]
        wk_sb = [wqk[:, 256:384], wqk[:, 384:512]]
        wv_sb = [wpB_sb[:, 256 * i:256 * (i + 1)] for i in range(2)]
        wo_sb = [wpB_sb[:, 512 + 256 * i:512 + 256 * (i + 1)] for i in range(4)]
        bq_sb = bp_sb[:, 0:1]
        bk_sb = bp_sb[:, 1:2]
        bo_sb = [bp_sb[:, 4 + i:5 + i] for i in range(2)]

        # PE warm-up: the clock gate needs ~3.4us of sustained matmul activity
        # to lift the PE from 1.2 to 2.4 GHz; burn the DMA-bound preamble.
        wu_ps = pe_pool.tile([128, 512], f32, name="wu_ps", tag="e")
        for r in range(8):
            nc.tensor.matmul(wu_ps, warm[:, 0:128], warm,
                             start=(r == 0), stop=(r == 7))

        def xq(i, col0, width):
            """AP into the query half (cols 0:2048) at column col0."""
            c = col0 // 1024
            return xn0[i][c][:, col0 % 1024:col0 % 1024 + width]

        def xkey(i, col0, width):
            """AP into the key range (cols 0:4096) at column col0."""
            if col0 < 2048:
                return xq(i, col0, width)
            return xn1[i][:, col0 - 2048:col0 - 2048 + width]

        # ---- all convs upfront (DMAs land early with parallel issue) ----
        q_sb = big.tile([128, M], f16, name="q_sb")
        for qb in range(M // 512):
            q_ps = pe_pool.tile([128, 512], f32, name=f"qps{qb}", tag="e")
            for ch in range(2):
                nc.tensor.matmul(q_ps, wq_sb[ch], xq(ch, qb * 512, 512),
                                 start=(ch == 0), stop=(ch == 1))
            nc.vector.tensor_scalar_add(q_sb[:, qb * 512:(qb + 1) * 512],
                                        q_ps, bq_sb)
        k_c = [big.tile([128, 512], f16, name=f"kc{kb}") for kb in range(N // 512)]
        # vt8: [128 keys, 32 subtiles, 256 ch] e4m3; DoubleRow stationary for
        # (pair p, ch) = vt8[:, 2p:2p+2, 128ch:128ch+128]
        vt8 = big.tile([128, NSUB, C], f8e4, name="vt8")

        def emit_kc(kb):
            k_ps = pe_pool.tile([128, 512], f32, name=f"kps{kb}", tag="e")
            for ch in range(2):
                nc.tensor.matmul(k_ps, wk_sb[ch], xkey(ch, kb * 512, 512),
                                 start=(ch == 0), stop=(ch == 1))
            nc.vector.tensor_scalar_add(k_c[kb], k_ps, bk_sb)

        def emit_vt(ns):
            vt_ps = pe_pool.tile([128, C], f32, name=f"vtps{ns}", tag="e")
            for ch in range(2):
                nc.tensor.matmul(vt_ps, xkey(ch, ns * 128, 128),
                                 wv_sb[ch], start=(ch == 0), stop=(ch == 1))
            nc.vector.tensor_copy(vt8[:, ns, :], vt_ps)

        # k/vT convs for the first few pairs run upfront; the rest are
        # interleaved into superblock 0's pair loop (they only gate later
        # pairs, and this keeps the PE busy while ScalarE works)
        for kb in range(2):
            emit_kc(kb)
        for ns in range(8):
            emit_vt(ns)

        # ---- attention: 4 m-superblocks of 512 query rows ----
        e_tiles = {}

        def emit_E4(sb, p):
            """Pairs p and p+1 of superblock sb as a 4-tile quad at row groups
            (0,32,64,96): the 128<->32-row mode-switch drain is paid once per
            TWO pairs.  k_c/q_sb carry 4 row-replicas of the 32 channels."""
            m0 = sb * SB
            eA = pe2.tile([128, 2, SB], f32, name=f"e_{sb}_{p}", tag="e2")
            eB = pe2.tile([128, 2, SB], f32, name=f"e_{sb}_{p + 1}", tag="e2")
            for j in range(4):
                i = 2 * p + j
                dst = (eA if j < 2 else eB)[:, j % 2, :]
                nc.tensor.matmul(dst,
                                 k_c[i // 4][32 * j:32 * (j + 1),
                                             (i % 4) * 128:(i % 4 + 1) * 128],
                                 q_sb[32 * j:32 * (j + 1), m0:m0 + SB],
                                 start=True, stop=True,
                                 tile_position=(32 * j, 0))
            e_tiles[(sb, p)] = eA
            e_tiles[(sb, p + 1)] = eB

        def s_eng(sb, p):
            """Which engine accumulates pair p's softmax-sum contribution.
            The PE's DoubleRow ones-matmul is cheapest but the PE is the
            bottleneck; sb0's loop also carries the k/vT convs, so its s
            work goes entirely to DVE/Pool (Pool is slow: ~2.2us per add)."""
            if sb == 0:
                return "pool" if p % 3 == 2 else "dve"
            return ("pe", "dve", "pool")[p % 3]

        def run_superblock(sb, tail_cbs):
            """Emit one superblock's n-pair loop.  tail_cbs is a dict
            {p: callback} of the previous superblock's tail pieces, spread
            across p positions so no engine queue head ever has a long
            unsatisfied dependency."""
            m0 = sb * SB
            out2 = [pacc.tile([128, SB], f32, name=f"out2_{sb}_{ch}", tag="out2")
                    for ch in range(2)]
            s_bc_box = []
            pe_pending = []
            saccs = {}    # engine -> [128,2,SB] f16 partial-sum tile
            started = []  # non-empty once the s_bc psum group is open

            def s_matmul(exp8):
                nc.tensor.matmul(s_bc_box[0], ones8, exp8,
                                 start=not started, stop=False, perf_mode=DR)
                started.append(True)

            if (sb, 0) not in e_tiles:   # sb 0; later sbs are prefetched
                emit_E4(sb, 0)
            for p in range(NP):
                if p % 2 == 1 and p + 1 < NP:
                    emit_E4(sb, p + 1)
                elif p == NP - 1 and sb < NSB - 1:
                    emit_E4(sb + 1, 0)   # cross-superblock prefetch
                # keep the PE's f16 ops (E4 quads, convs) and fp8-DoubleRow
                # ops (PV, s) in contiguous runs: every extra mode switch
                # costs a pipeline drain
                if sb == 0 and 2 * p + 8 < NSUB:
                    emit_vt(2 * p + 8)
                    emit_vt(2 * p + 9)
                if sb == 0 and (p + 1) % 2 == 0 and (p + 1) // 2 + 1 < N // 512:
                    emit_kc((p + 1) // 2 + 1)
                e_ps = e_tiles.pop((sb, p))
                exp8 = expp.tile([128, 2, SB], f8e5, name=f"exp_{sb}_{p}",
                                 tag="exp")
                nc.scalar.activation(exp8, e_ps, Exp, bias=ebias, scale=4.0)
                for ch in range(2):
                    nc.tensor.matmul(out2[ch],
                                     vt8[:, 2 * p:2 * p + 2,
                                         ch * 128:(ch + 1) * 128],
                                     exp8, start=(p == 0), stop=(p == NP - 1),
                                     perf_mode=DR)
                # softmax denominator, split across engines.  PE-share pairs
                # go through a DoubleRow ones-matmul into s_bc [128,SB] f32
                # PSUM (deferred past p==3 so the ps bank is free: the prev
                # superblock's s is read by Ln at p==1).  DVE/Pool-share
                # pairs accumulate fp16 partials, folded into s_bc by two
                # f16 matmuls at the end of the loop.
                eng = s_eng(sb, p)
                if eng == "pe":
                    pe_pending.append(exp8)
                    if p >= 3:
                        if not s_bc_box:
                            s_bc_box.append(ps_pool.tile(
                                [128, SB], f32, name=f"s_{sb}", tag="ps"))
                        for e8 in pe_pending:
                            s_matmul(e8)
                        pe_pending.clear()
                else:
                    op = (nc.vector if eng == "dve" else nc.gpsimd)
                    if eng not in saccs:
                        t = big.tile([128, 2, SB], f16, name=f"sacc_{sb}_{eng}")
                        saccs[eng] = t
                        op.tensor_copy(t, exp8)
                    else:
                        op.tensor_add(saccs[eng], saccs[eng], exp8)
                cb = tail_cbs.get(p) if tail_cbs else None
                if cb is not None:
                    cb()

            def fold():
                # fold the DVE/Pool fp16 partials into s_bc via f16 ones
                # matmuls.  Deferred into the NEXT superblock's loop (just
                # before Ln) so the PE never stalls on the accumulators'
                # final elementwise add at the boundary.
                if not s_bc_box:
                    s_bc_box.append(ps_pool.tile([128, SB], f32,
                                                 name=f"s_{sb}", tag="ps"))
                folds = [saccs[e] for e in ("dve", "pool") if e in saccs]
                for fi, acc in enumerate(folds):
                    for jj in range(2):
                        nc.tensor.matmul(s_bc_box[0], ones128f, acc[:, jj, :],
                                         start=(not started and fi == 0
                                                and jj == 0),
                                         stop=(fi == len(folds) - 1
                                               and jj == 1))
                started.append(True)

            out2sb = [big.tile([128, SB], f16, name=f"out2sb_{sb}_{ch}")
                      for ch in range(2)]
            nc.vector.tensor_copy(out2sb[0], out2[0])
            nc.vector.tensor_copy(out2sb[1], out2[1])
            return s_bc_box, fold, out2sb

        def make_tail(sb, s_bc_box, fold, out2sb, last):
            """Tail pieces for superblock sb, as {p: callback} for the NEXT
            superblock's loop (or run inline after the last superblock).
            y = (Wo1T.T @ out2sb) * inv + (Wo2T.T @ xq + cvec)."""
            m0 = sb * SB
            box = {}

            def schain():   # p==1: 1/s = exp(-ln s), broadcast on all rows
                fold()
                ln_s = scp.tile([128, SB], f32, name=f"ln_{sb}", tag="ln_s")
                nc.scalar.activation(ln_s, s_bc_box[0], Ln, bias=zbias)
                inv16 = scp.tile([128, SB], f16, name=f"inv16_{sb}",
                                 tag="inv16")
                nc.scalar.activation(inv16, ln_s, Exp, bias=zbias, scale=-1.0)
                box["inv_sb"] = inv16

            # the last superblock's tail runs after the loop when the e-pair
            # PSUM banks are free: back ya/yx with the pe2 pool there so the
            # PE never waits on a DVE consumer between tail matmuls
            tpool, ttag = (pe2, "e2") if last else (pe_pool, "e")

            def ya(cho):    # p==4, 5
                a_ps = tpool.tile([128, SB], f32, name=f"ya_{sb}_{cho}",
                                  tag=ttag)
                for j in range(2):
                    nc.tensor.matmul(a_ps, wo_sb[j][:, cho * 128:(cho + 1) * 128],
                                     out2sb[j], start=(j == 0), stop=(j == 1))
                yt = scp.tile([128, SB], f16, name=f"yt_{sb}_{cho}", tag="yt")
                nc.vector.tensor_mul(yt, a_ps, box["inv_sb"])
                box[f"yt{cho}"] = yt

            def yx(cho):    # p==6, 7
                x_ps = tpool.tile([128, SB], f32, name=f"yx_{sb}_{cho}",
                                  tag=ttag)
                for j in range(2):
                    nc.tensor.matmul(x_ps,
                                     wo_sb[2 + j][:, cho * 128:(cho + 1) * 128],
                                     xq(j, m0, SB), start=(j == 0), stop=False)
                nc.tensor.matmul(x_ps, cv_sb[:, cho * 128:(cho + 1) * 128],
                                 ones_row, start=False, stop=True)
                y_sb = yp.tile([128, SB], f16, name=f"y_{sb}_{cho}", tag="ysb")
                nc.vector.tensor_add(y_sb, box[f"yt{cho}"], x_ps)
                if last and cho == 0:
                    nc.sync.dma_start(out=out[0:128, m0:m0 + SB], in_=y_sb)
                elif last:
                    nc.gpsimd.dma_start(out=out[128:256, m0:m0 + SB], in_=y_sb)
                else:
                    nc.sync.dma_start(
                        out=out[cho * 128:(cho + 1) * 128, m0:m0 + SB],
                        in_=y_sb)

            box_cb = {1: schain, 4: lambda: ya(0), 5: lambda: ya(1),
                      6: lambda: yx(0), 7: lambda: yx(1)}
            return box_cb

        tail_cbs = None
        for sb in range(NSB):
            s_bc_box, fold, out2sb = run_superblock(sb, tail_cbs)
            if sb < NSB - 1:
                tail_cbs = make_tail(sb, s_bc_box, fold, out2sb, last=False)
            else:
                # last superblock: run the tail inline, same piece order
                cbs = make_tail(sb, s_bc_box, fold, out2sb, last=True)
                for p in (1, 4, 5, 6, 7):
                    cbs[p]()

    return nc


_cached_nc = None


def _make_in_maps(x, Wq, bq, Wk, bk, Wv, bv, Wo, bo):
    f16 = np.float16
    f32 = np.float32
    xf = np.ascontiguousarray(np.asarray(x, dtype=f32).reshape(4, C, N))
    # q/k weights are replicated 4x along the free dim ON-CHIP; host ships
    # the unreplicated [256, 32] transposes (k scaled by 1/4: the exp
    # activation's scale=4 recovers it exactly).
    wqT = np.ascontiguousarray(np.asarray(Wq, dtype=f32).T).astype(f16)
    wkT = (np.ascontiguousarray(np.asarray(Wk, dtype=f32).T) / 4.0).astype(f16)
    wvT = np.ascontiguousarray(np.asarray(Wv, dtype=f32).T).astype(f16)
    woT = np.ascontiguousarray(np.asarray(Wo, dtype=f32).T).astype(f16)
    # wpA [128, 128]: wq0u|wq1u|wk0u|wk1u
    wpA = np.ascontiguousarray(np.concatenate(
        [wqT[0:128], wqT[128:256], wkT[0:128], wkT[128:256]], axis=1))
    # wpB [128, 1536]: wv0|wv1|wo0..3
    wpB = np.ascontiguousarray(np.concatenate(
        [wvT[0:128], wvT[128:256],
         woT[0:128], woT[128:256], woT[256:384], woT[384:512]], axis=1))
    # bpack [128, 8] f32: bq|bk|bv0|bv1|bo0|bo1|pad (bq/bk replicated 4x)
    bq2 = np.tile(np.asarray(bq, dtype=f32).reshape(CQK, 1), (4, 1))
    bk2 = np.tile(np.asarray(bk, dtype=f32).reshape(CQK, 1) / 4.0, (4, 1))
    bv2 = np.asarray(bv, dtype=f32).reshape(C, 1)
    bo2 = np.asarray(bo, dtype=f32).reshape(C, 1)
    bpack = np.zeros((128, 8), dtype=f32)
    bpack[:, 0:1] = bq2
    bpack[:, 1:2] = bk2
    bpack[:, 2:3] = bv2[0:128]
    bpack[:, 3:4] = bv2[128:256]
    bpack[:, 4:5] = bo2[0:128]
    bpack[:, 5:6] = bo2[128:256]
    # cvec = bo + Wo1.T @ bv (softmax rows sum to 1, so the bv add is exact)
    cvec = (np.asarray(bo, dtype=np.float64)
            + np.asarray(Wo, dtype=np.float64)[:, :C] @ np.asarray(bv, np.float64))
    cpack = np.ascontiguousarray(cvec.reshape(1, 256)).astype(f16)
    in_maps = []
    for core in range(8):
        b, h = core // 2, core % 2
        # permute keys so this core's query half comes first; softmax and
        # the PV sum are invariant to key order
        if h == 0:
            xn_a = xf[b].astype(f16)
        else:
            xn_a = np.ascontiguousarray(
                np.concatenate([xf[b][:, M:], xf[b][:, :M]], axis=1)).astype(f16)
        in_maps.append({
            "xn": xn_a, "wpA": wpA, "wpB": wpB, "bpack": bpack, "cpack": cpack,
        })
    return in_maps


def kernel_run(inputs, trace=False, trace_kwargs=None):
    """Run on 8 cores; returns (full_output, BassKernelResults)."""
    global _cached_nc
    _install_wait_split()
    from concourse.bass_utils import run_bass_kernel_spmd

    if _cached_nc is None:
        _cached_nc = _build_nc()
    in_maps = _make_in_maps(**inputs)
    res = run_bass_kernel_spmd(_cached_nc, in_maps, core_ids=list(range(8)),
                               trace=trace, **(trace_kwargs or {}))
    y = np.empty((4, C, N), dtype=np.float32)
    for core in range(8):
        b, h = core // 2, core % 2
        y[b][:, h * M:(h + 1) * M] = res.results[core]["out"].astype(np.float32)
    return y.reshape(4, C, 64, 64), res


def kernel(**inputs):
    y, _ = kernel_run(inputs, trace=False)
    return y
